# revision 65
# baseline (speedup 1.0000x reference)
"""Trainium2 Bass kernel for nn_Attention (B=1, S=2048, D=4096, H=32, KVH=8).

Sharding: tensor-parallel over heads across 8 cores (4 Q heads + 1 KV head
per core). Each core computes QKV projections for its heads from the full
x^T, applies RoPE, runs causal flash attention in transposed layout, then a
per-head AllToAll redistributes attention outputs so each core computes the
final out-projection rows for its 1/8 of the sequence against the full wo.
The host concatenates the per-core row blocks.

Matmuls run in bf16; accumulation and softmax bookkeeping stay fp32 in PSUM.
Perf structure (the chip power-throttles the PE to 13/16 clock under
sustained matmul load, so streamed PE columns are the binding resource):
- a tiny warmup AllToAll at kernel start absorbs the ~50us first-collective
  firmware cost, so the real per-head a2a chain runs at ~13-20us each and
  hides behind attention/out-projection compute;
- attention PV and rowsum matmuls skip the fully-masked query columns of
  causal-diagonal tiles, and the mask multiply touches only the partial
  region;
- a cross-block software pipeline (PV pairs trail the score stream by two
  pairs across block boundaries) keeps the PE dense through block and head
  transitions;
- the first 3 wo blocks prefetch during attention on the ACT DMA queue;
  phase 3 runs a head-wave schedule (h0/h1 for 4 no-blocks, then h2, then
  h3) so each head's a2a lands before its wave starts.
"""

import sys

for _p in ("/opt/trn_rl_repo",):
    if _p not in sys.path:
        sys.path.insert(0, _p)

from contextlib import ExitStack
from math import sqrt

import numpy as np

import concourse.bass as bass
import concourse.tile as tile
from concourse import bacc, mybir
from concourse.masks import make_identity

F32 = mybir.dt.float32
F32R = mybir.dt.float32r
BF16 = mybir.dt.bfloat16

FULL_CFG = dict(S=2048, D=4096, H=32, KVH=8, HD=128, NB=512, n_cores=8,
                dt="bf16", pre_n=3)


def build_bass(cfg):
    """Build the SPMD per-core Bass program. Same program on every core; all
    per-core differences live in the input data."""
    S, D, H, KVH, HD = cfg["S"], cfg["D"], cfg["H"], cfg["KVH"], cfg["HD"]
    NB, NC = cfg["NB"], cfg["n_cores"]
    MM = BF16 if cfg.get("dt", "bf16") == "bf16" else F32R
    HPC = H // NC          # q heads per core
    KC = D // 128          # contraction chunks for projections
    NSB = S // NB          # seq blocks
    NKJ = NB // 128        # kj 128-blocks per seq block
    R = S // NC            # output rows per core
    MT = R // 128          # output row tiles
    NO = D // 512          # out-proj column blocks
    PRE_N = min(cfg.get("pre_n", 0), NO)
    scale = 1.0 / sqrt(HD)

    assert NB % R == 0 or R % NB == 0
    PIECES = max(1, NB // R)   # a2a pieces per (head, seq block)

    nc = bacc.Bacc(
        "TRN2",
        target_bir_lowering=False,
        debug=False,
        enable_asserts=False,
        num_devices=NC,
    )

    # pre-blocked host layouts: big DMAs read contiguous per-partition lines
    xT = nc.dram_tensor("xT", [NSB, 128, KC, NB], MM, kind="ExternalInput").ap()
    wq = nc.dram_tensor("wq", [HPC, 128, KC, HD], MM, kind="ExternalInput").ap()
    wk = nc.dram_tensor("wk", [128, KC, HD], MM, kind="ExternalInput").ap()
    wv = nc.dram_tensor("wv", [128, KC, HD], MM, kind="ExternalInput").ap()
    wo = nc.dram_tensor("wo", [NO, 128, H, 512], MM, kind="ExternalInput").ap()
    CS = BF16 if MM == BF16 else F32
    cosT = nc.dram_tensor("cosT", [HD // 2, S], CS, kind="ExternalInput").ap()
    sinT = nc.dram_tensor("sinT", [HD // 2, S], CS, kind="ExternalInput").ap()
    out = nc.dram_tensor("out", [R, D], F32, kind="ExternalOutput").ap()

    with tile.TileContext(nc) as tc, ExitStack() as ctx:
        dram = ctx.enter_context(tc.tile_pool(name="dram", bufs=1, space="DRAM"))
        a2a_in = [dram.tile([NC, 128, R], MM, tag=f"ain{h}", name=f"ain{h}")
                  for h in range(HPC)]
        a2a_out = [dram.tile([NC, 128, R], MM, tag=f"aout{h}", name=f"aout{h}")
                   for h in range(HPC)]
        # tiny warmup collective: absorbs the ~50us first-collective firmware
        # cost while the PE streams QKV, so the real a2a chain runs at ~13us
        warm_i = dram.tile([NC, 64], MM, tag="warm_i", name="warm_i")
        warm_o = dram.tile([NC, 64], MM, tag="warm_o", name="warm_o")

        qkvp = ctx.enter_context(tc.tile_pool(name="qkv", bufs=1))
        QT = [qkvp.tile([128, S], MM, tag=f"qt{h}", name=f"QT{h}")
              for h in range(HPC)]
        KT = qkvp.tile([128, S], MM, tag="kt", name="KT")
        Vn = qkvp.tile([128, S], MM, tag="vn", name="Vn")
        ones = qkvp.tile([128, 128], MM, tag="ones", name="ones")
        masks = [qkvp.tile([128, NB], MM, tag=f"mask{dd}", name=f"mask{dd}")
                 for dd in range(NKJ)]
        with tc.tile_pool(name="init", bufs=1) as initp:
            ones_f = initp.tile([128, 128], F32, tag="ones_f", name="ones_f")
            nc.vector.memset(ones_f, 1.0)
            nc.vector.tensor_copy(ones, ones_f)
            # causal masks for the 4 diagonal offsets (DVE multiplies, bf16)
            onb_f = initp.tile([128, NB], F32, tag="onb_f", name="onb_f")
            onb = initp.tile([128, NB], MM, tag="onb", name="onb")
            nc.vector.memset(onb_f, 1.0)
            nc.vector.tensor_copy(onb, onb_f)
            for dd in range(NKJ):
                nc.gpsimd.affine_select(
                    out=masks[dd], in_=onb, compare_op=mybir.AluOpType.is_ge,
                    fill=0.0, base=-dd * 128, pattern=[[1, NB]],
                    channel_multiplier=-1,
                )
        nc.gpsimd.collective_compute(
            "AllToAll",
            mybir.AluOpType.bypass,
            replica_groups=[list(range(NC))],
            ins=[warm_i.opt()],
            outs=[warm_o.opt()],
        )

        # ============= Phase 1: QKV projection + RoPE =============
        with (
            tc.tile_pool(name="wgt", bufs=1) as wgt,
            tc.tile_pool(name="trig", bufs=1) as trig,
            tc.tile_pool(name="xtp", bufs=8) as xtp,
            tc.tile_pool(name="vt_sb", bufs=1) as vtp,
            tc.tile_pool(name="pj_ps", bufs=2, space="PSUM") as pjps,
            tc.tile_pool(name="tps", bufs=2, space="PSUM") as tps,
            tc.tile_pool(name="rope_t", bufs=4) as ropep,
        ):
            hw_ = HD // 2
            cos_sb = trig.tile([hw_, S], CS, tag="cos", name="cos_sb")
            sin_sb = trig.tile([hw_, S], CS, tag="sin", name="sin_sb")
            ident = trig.tile([128, 128], MM, tag="ident", name="ident")
            make_identity(nc, ident)

            wq_sb = wgt.tile([128, HPC, KC, HD], MM, tag="wq", name="wq_sb")
            wk_sb = wgt.tile([128, KC, HD], MM, tag="wk", name="wk_sb")
            wv_sb = wgt.tile([128, KC, HD], MM, tag="wv", name="wv_sb")
            # first weights split across two cold DMA queues so the first
            # projection matmuls start as early as possible
            # DMA order matches compute order k, q0..q3, v: wv arrives last
            # since the v projection now runs last in each block
            nc.scalar.dma_start(out=wk_sb[:, :KC // 2], in_=wk[:, :KC // 2])
            nc.scalar.dma_start(out=wk_sb[:, KC // 2:], in_=wk[:, KC // 2:])
            nc.scalar.dma_start(out=wq_sb[:, 0], in_=wq[0])
            nc.scalar.dma_start(out=cos_sb, in_=cosT)
            nc.scalar.dma_start(out=sin_sb, in_=sinT)
            for h in range(1, HPC):
                nc.scalar.dma_start(out=wq_sb[:, h], in_=wq[h])
            nc.scalar.dma_start(out=wv_sb, in_=wv)

            VT_sb = vtp.tile([128, S], MM, tag="vt", name="VT_sb")

            # v LAST: its PSUM drain is a fast ACT copy (vs the 6-op DVE rope
            # chain), so the last block's banks free ~1us after its final
            # matmul and the attention scores start without a PSUM-WAR wait
            outs = [(lambda k: wk_sb[:, k, :], "k", 0)]
            for h in range(HPC):
                outs.append((lambda k, h=h: wq_sb[:, h, k, :], "q", h))
            outs.append((lambda k: wv_sb[:, k, :], "v", 0))

            KH = KC // 4
            for n in range(NSB):
                nsl = slice(n * NB, (n + 1) * NB)
                xh = []
                for half in range(4):
                    xb = xtp.tile([128, KH, NB], MM, tag="x", name="xb")
                    if n == 0 and half == 0:
                        # split the very first x chunk so the first matmul
                        # (which only needs k-chunk 0) starts ~4us earlier
                        # on the cold DMA queue
                        h2_ = KH // 2
                        nc.sync.dma_start(out=xb[:, :h2_],
                                          in_=xT[n, :, :h2_, :])
                        nc.sync.dma_start(out=xb[:, h2_:],
                                          in_=xT[n, :, h2_:KH, :])
                    else:
                        nc.sync.dma_start(
                            out=xb, in_=xT[n, :, half * KH:(half + 1) * KH, :])
                    xh.append(xb)
                # output-major with a rotating 3-deep PSUM tag: long
                # same-bank runs keep the PE warm while earlier outputs drain
                for wsel, kind, h in outs:
                    ps = pjps.tile([128, NB], F32, tag="pj", name="ps")
                    for k in range(KC):
                        nc.tensor.matmul(
                            ps, lhsT=wsel(k),
                            rhs=xh[k // KH][:, k % KH, :],
                            start=(k == 0), stop=(k == KC - 1),
                        )
                    if kind == "v":
                        nc.scalar.copy(VT_sb[:, nsl], ps)
                        for t in range(n * NB // 128, (n + 1) * NB // 128):
                            tsl = slice(t * 128, (t + 1) * 128)
                            pst = tps.tile([128, 128], MM, tag="tp",
                                           name="pst")
                            nc.tensor.transpose(pst, VT_sb[:, tsl], ident)
                            nc.scalar.copy(Vn[:, tsl], pst)
                    else:
                        dst = KT if kind == "k" else QT[h]
                        E, O = ps[0:hw_, :], ps[hw_:2 * hw_, :]
                        c, s = cos_sb[:, nsl], sin_sb[:, nsl]
                        t1 = ropep.tile([hw_, NB], F32, tag="rt", name="t1")
                        t2 = ropep.tile([hw_, NB], F32, tag="rt", name="t2")
                        nc.vector.tensor_mul(t1, E, c)
                        nc.vector.tensor_mul(t2, O, s)
                        nc.vector.tensor_sub(dst[0:hw_, nsl], t1, t2)
                        t3 = ropep.tile([hw_, NB], F32, tag="rt", name="t3")
                        t4 = ropep.tile([hw_, NB], F32, tag="rt", name="t4")
                        nc.vector.tensor_mul(t3, E, s)
                        nc.vector.tensor_mul(t4, O, c)
                        nc.vector.tensor_add(dst[hw_:, nsl], t3, t4)


        # P3 SBUF pools open before P2 so their DMAs can prefetch during it
        pcp = ctx.enter_context(tc.tile_pool(name="pc_sb", bufs=1))
        wop = ctx.enter_context(tc.tile_pool(name="wo_sb", bufs=4))
        obp = ctx.enter_context(tc.tile_pool(name="ob_sb", bufs=3))
        piece = {}

        wo_pre = {}

        def get_wob(no):
            if no in wo_pre:
                return wo_pre[no]
            wob = wop.tile([128, H, 512], MM, tag="wob", name="wob")
            q2 = H // 2
            for qq in range(0, H, q2):
                nc.scalar.dma_start(out=wob[:, qq:qq + q2],
                                    in_=wo[no, :, qq:qq + q2])
            wo_pre[no] = wob
            return wob

        # prefetch the first 3 wo blocks: the ACT queue reaches these DMAs
        # before the phase-2 exps, so they stream during attention
        for no in range(PRE_N):
            get_wob(no)

        # ============= Phase 2: causal flash attention =============
        with (
            tc.tile_pool(name="sc_ps", bufs=2, space="PSUM") as scps,
            tc.tile_pool(name="o_ps", bufs=2, space="PSUM") as ops_,
            tc.tile_pool(name="rs_ps", bufs=2, space="PSUM") as rsps,
            tc.tile_pool(name="exp_sb", bufs=3) as exps,
            tc.tile_pool(name="att_sb", bufs=4) as atts,
        ):
            # unified cross-block software pipeline: PV pairs trail the score
            # stream by 2 pairs ACROSS block boundaries, and each block's
            # rowsum matmul + normalize + piece stores are emitted once its
            # last PV pops — so the PE never drains at a block boundary
            inflight = []

            def finish_block(b):
                rcp = atts.tile([128, NB], F32, tag="rcp", name="rcp")
                nc.vector.reciprocal_approx_fast(rcp, b["rs_ps"])
                o_sb = atts.tile([128, NB], MM, tag="osb", name="o_sb")
                nc.vector.tensor_mul(o_sb, b["o_ps"], rcp)
                hh, nn = b["h"], b["n"]
                for jj in range(PIECES):
                    piece_idx = nn * PIECES + jj
                    nc.sync.dma_start(
                        out=a2a_in[hh][piece_idx],
                        in_=o_sb[:, jj * R:(jj + 1) * R],
                    )
                if nn == NSB - 1:
                    nc.gpsimd.collective_compute(
                        "AllToAll",
                        mybir.AluOpType.bypass,
                        replica_groups=[list(range(NC))],
                        ins=[a2a_in[hh].opt()],
                        outs=[a2a_out[hh].opt()],
                    )
                    # pull head hh-1's redistributed pieces into SBUF now;
                    # its a2a completed during head hh's attention
                    if hh >= 1:
                        for i in range(NC):
                            t = pcp.tile([128, R], MM, tag=f"pc{hh - 1}_{i}",
                                         name=f"pc{hh - 1}_{i}")
                            nc.sync.dma_start(out=t, in_=a2a_out[hh - 1][i])
                            piece[(hh - 1, i)] = t

            def emit_pv(entry):
                pair, b = entry
                for (j, pex) in pair:
                    psl = slice(j * 128, (j + 1) * 128)
                    d = j - b["n"] * NKJ
                    last = j == b["last_j"]
                    if d > 0:
                        # diagonal tile: query cols < d*128 are fully masked
                        # (pex is 0 there) — skip streaming them
                        qs = d * 128
                        nc.tensor.matmul(b["o_ps"][:, qs:],
                                         lhsT=Vn[:, psl], rhs=pex[:, qs:],
                                         start=False, stop=last)
                        nc.tensor.matmul(b["rs_ps"][:, qs:], lhsT=ones,
                                         rhs=pex[:, qs:],
                                         start=False, stop=last)
                    else:
                        nc.tensor.matmul(b["o_ps"], lhsT=Vn[:, psl], rhs=pex,
                                         start=(j == 0), stop=last)
                        nc.tensor.matmul(b["rs_ps"], lhsT=ones, rhs=pex,
                                         start=(j == 0), stop=last)
                b["npv"] -= 1
                if b["npv"] == 0:
                    finish_block(b)

            def pump(maxlag):
                while len(inflight) > maxlag:
                    emit_pv(inflight.pop(0))

            for h in range(HPC):
                for n in range(NSB):
                    nsl = slice(n * NB, (n + 1) * NB)
                    nkj = (n + 1) * NKJ
                    b = dict(
                        o_ps=ops_.tile([128, NB], F32, tag="o", name="o_ps"),
                        rs_ps=rsps.tile([128, NB], F32, tag="rs",
                                        name="rs_ps"),
                        h=h, n=n, last_j=nkj - 1, npv=nkj // 2,
                    )
                    q_rhs = QT[h][:, nsl]

                    for jp in range(nkj // 2):
                        sc = scps.tile([128, 2, NB], F32, tag="sc",
                                       name="sc")
                        ex = exps.tile([128, 2, NB], MM, tag="ex", name="ex")
                        pair = []
                        for half in range(2):
                            j = 2 * jp + half
                            jsl = slice(j * 128, (j + 1) * 128)
                            nc.tensor.matmul(sc[:, half], lhsT=KT[:, jsl],
                                             rhs=q_rhs, start=True,
                                             stop=True)
                            pair.append((j, ex[:, half]))
                        pump(2)
                        nc.scalar.activation(
                            ex.rearrange("p a b -> p (a b)"),
                            sc.rearrange("p a b -> p (a b)"),
                            mybir.ActivationFunctionType.Exp, scale=scale,
                        )
                        for half in range(2):
                            j = 2 * jp + half
                            d = j - n * NKJ
                            if d >= 0:  # overlaps the causal diagonal
                                # columns >= (d+1)*128 keep their value
                                # (mask is 1 there) — multiply only the
                                # zero/partial region
                                w = (d + 1) * 128
                                nc.vector.tensor_mul(ex[:, half, :w],
                                                     ex[:, half, :w],
                                                     masks[d][:, :w])
                        inflight.append((pair, b))
            pump(0)

            for i in range(NC):
                t = pcp.tile([128, R], MM, tag=f"pc{HPC - 1}_{i}",
                             name=f"pc{HPC - 1}_{i}")
                nc.sync.dma_start(out=t, in_=a2a_out[HPC - 1][i])
                piece[(HPC - 1, i)] = t


        # ============= Phase 3: out-projection =============
        with tc.tile_pool(name="op_ps", bufs=2, space="PSUM") as opps:

            def emit_mms(pso, wob, heads, first):
                for m in range(MT):
                    f = first
                    for h in heads:
                        for i in range(NC):
                            g = i * HPC + h
                            last = (h == HPC - 1) and (i == NC - 1)
                            nc.tensor.matmul(
                                pso[m],
                                lhsT=piece[(h, i)][:, m * 128:(m + 1) * 128],
                                rhs=wob[:, g, :], start=f, stop=last,
                            )
                            f = False
                return False

            def drain(pso, no):
                osl = slice(no * 512, (no + 1) * 512)
                for m in range(MT):
                    ob = obp.tile([128, 512], F32, tag="ob", name="ob")
                    nc.vector.tensor_copy(ob, pso[m])
                    nc.sync.dma_start(
                        out=out[m * 128:(m + 1) * 128, osl], in_=ob)

            # wave schedule over the first DEFER no-blocks: heads {0,1}
            # (pair-0 a2a, done mid-attention) first, then h2 and h3 whose
            # pair-1 a2a lands while the h01 wave streams
            DEFER = min(NO, 4) if HPC > 1 else 0
            state = {}
            for no in range(DEFER):
                wob = get_wob(no)
                pso = [opps.tile([128, 512], F32, tag=f"po{m}", name=f"pso{m}",
                                 bufs=max(2, DEFER)) for m in range(MT)]
                state[no] = (pso, wob)
                emit_mms(pso, wob, [0, 1], True)
            for no in range(DEFER):
                pso, wob = state[no]
                emit_mms(pso, wob, [2], False)
            for no in range(DEFER):
                pso, wob = state[no]
                emit_mms(pso, wob, [3], False)
                drain(pso, no)
            for no in range(DEFER, NO):
                wob = get_wob(no)
                pso = [opps.tile([128, 512], F32, tag=f"po{m}", name=f"pso{m}",
                                 bufs=max(2, DEFER)) for m in range(MT)]
                emit_mms(pso, wob, range(HPC), True)
                drain(pso, no)

    nc.compile()
    return nc


def prep_inputs(cfg, x, wq, wk, wv, wo, freqs_cos, freqs_sin):
    """Host-side sharding/layout prep. Returns list of per-core input dicts."""
    S, D, H, KVH, HD, NC = (cfg["S"], cfg["D"], cfg["H"], cfg["KVH"], cfg["HD"],
                            cfg["n_cores"])
    HPC = H // NC
    if cfg.get("dt", "bf16") == "bf16":
        import ml_dtypes
        mdt = ml_dtypes.bfloat16
    else:
        mdt = np.float32
    x = np.asarray(x, np.float32).reshape(S, D)
    wq = np.asarray(wq, np.float32)
    wk = np.asarray(wk, np.float32)
    wv = np.asarray(wv, np.float32)
    wo = np.asarray(wo, np.float32)
    cos = np.asarray(freqs_cos, np.float32)
    sin = np.asarray(freqs_sin, np.float32)

    NSB, NB = S // cfg["NB"], cfg["NB"]
    KC = D // 128
    NO = D // 512
    # x blocked: xp[n, p, k, s] = x[n*NB+s, k*128+p]
    xp = np.ascontiguousarray(
        x.reshape(NSB, NB, KC, 128).transpose(0, 3, 2, 1)).astype(mdt)
    cosT = np.ascontiguousarray(cos.T).astype(mdt)      # [HD/2, S]
    sinT = np.ascontiguousarray(sin.T).astype(mdt)
    # wo blocked: wop_[no, p, g, m] = wo[g*128+p, no*512+m]
    wo_c = np.ascontiguousarray(
        wo.reshape(H, 128, NO, 512).transpose(2, 1, 0, 3)).astype(mdt)

    # de-interleave rope pairs: new col i <- 2i, new col i+HD/2 <- 2i+1
    idx = np.concatenate([np.arange(0, HD, 2), np.arange(1, HD, 2)])
    wq_p = wq.reshape(D, H, HD)[:, :, idx]
    wk_p = wk.reshape(D, KVH, HD)[:, :, idx]
    wv_r = wv.reshape(D, KVH, HD)

    def wblock(w):
        # [D, M] -> [128, KC, M] with wb[p, k, m] = w[k*128+p, m]
        M = w.shape[1]
        return np.ascontiguousarray(
            w.reshape(KC, 128, M).transpose(1, 0, 2)).astype(mdt)

    in_maps = []
    for c in range(NC):
        kv = c * KVH // NC
        in_maps.append(dict(
            xT=xp,
            wq=np.stack([wblock(np.ascontiguousarray(wq_p[:, c * HPC + h]))
                         for h in range(HPC)]),
            wk=wblock(np.ascontiguousarray(wk_p[:, kv])),
            wv=wblock(np.ascontiguousarray(wv_r[:, kv])),
            wo=wo_c,
            cosT=cosT,
            sinT=sinT,
        ))
    return in_maps


_CACHED = {}


def _get_nc(cfg_key=None):
    if "nc" not in _CACHED:
        _CACHED["nc"] = build_bass(FULL_CFG)
    return _CACHED["nc"]


def run_spmd(x, wq, wk, wv, wo, freqs_cos, freqs_sin, **spmd_kwargs):
    """Build (cached), run on 8 cores, return (full_output, BassKernelResults)."""
    from concourse.bass_utils import run_bass_kernel_spmd

    cfg = FULL_CFG
    NC = cfg["n_cores"]
    in_maps = prep_inputs(cfg, x, wq, wk, wv, wo, freqs_cos, freqs_sin)
    nc = _get_nc()
    res = run_bass_kernel_spmd(nc, in_maps, list(range(NC)), **spmd_kwargs)
    parts = [res.results[c]["out"] for c in range(NC)]
    full = np.concatenate(parts, axis=0)
    return full.reshape(1, cfg["S"], cfg["D"]).astype(np.float32), res


def kernel(x, wq, wk, wv, wo, freqs_cos, freqs_sin):
    out, _ = run_spmd(x, wq, wk, wv, wo, freqs_cos, freqs_sin)
    return out



# revision 67
# speedup vs baseline: 1.0315x; 1.0315x over previous
"""Trainium2 Bass kernel for nn_Attention (B=1, S=2048, D=4096, H=32, KVH=8).

Sharding: tensor-parallel over heads across 8 cores (4 Q heads + 1 KV head
per core). Each core computes QKV projections for its heads from the full
x^T, applies RoPE, runs causal flash attention in transposed layout, then a
per-head AllToAll redistributes attention outputs so each core computes the
final out-projection rows for its 1/8 of the sequence against the full wo.
The host concatenates the per-core row blocks.

Matmuls run in bf16; accumulation and softmax bookkeeping stay fp32 in PSUM.
Perf structure (the chip power-throttles the PE to 13/16 clock under
sustained matmul load, so streamed PE columns are the binding resource):
- a tiny warmup AllToAll at kernel start absorbs the ~50us first-collective
  firmware cost, so the real per-head a2a chain runs at ~13-20us each and
  hides behind attention/out-projection compute;
- attention PV and rowsum matmuls skip the fully-masked query columns of
  causal-diagonal tiles, and the mask multiply touches only the partial
  region;
- a cross-block software pipeline (PV pairs trail the score stream by two
  pairs across block boundaries) keeps the PE dense through block and head
  transitions;
- the first 3 wo blocks prefetch during attention on the ACT DMA queue;
  phase 3 runs a head-wave schedule (h0/h1 for 4 no-blocks, then h2, then
  h3) so each head's a2a lands before its wave starts.
"""

import sys

for _p in ("/opt/trn_rl_repo",):
    if _p not in sys.path:
        sys.path.insert(0, _p)

from contextlib import ExitStack
from math import sqrt

import numpy as np

import concourse.bass as bass
import concourse.tile as tile
from concourse import bacc, mybir
from concourse.masks import make_identity

F32 = mybir.dt.float32
F32R = mybir.dt.float32r
BF16 = mybir.dt.bfloat16

FULL_CFG = dict(S=2048, D=4096, H=32, KVH=8, HD=128, NB=512, n_cores=8,
                dt="bf16", pre_n=3)


def build_bass(cfg):
    """Build the SPMD per-core Bass program. Same program on every core; all
    per-core differences live in the input data."""
    S, D, H, KVH, HD = cfg["S"], cfg["D"], cfg["H"], cfg["KVH"], cfg["HD"]
    NB, NC = cfg["NB"], cfg["n_cores"]
    MM = BF16 if cfg.get("dt", "bf16") == "bf16" else F32R
    HPC = H // NC          # q heads per core
    KC = D // 128          # contraction chunks for projections
    NSB = S // NB          # seq blocks
    NKJ = NB // 128        # kj 128-blocks per seq block
    R = S // NC            # output rows per core
    MT = R // 128          # output row tiles
    NO = D // 512          # out-proj column blocks
    PRE_N = min(cfg.get("pre_n", 0), NO)
    scale = 1.0 / sqrt(HD)

    assert NB % R == 0 or R % NB == 0
    PIECES = max(1, NB // R)   # a2a pieces per (head, seq block)

    nc = bacc.Bacc(
        "TRN2",
        target_bir_lowering=False,
        debug=False,
        enable_asserts=False,
        num_devices=NC,
    )

    # pre-blocked host layouts: big DMAs read contiguous per-partition lines
    xT = nc.dram_tensor("xT", [NSB, 128, KC, NB], MM, kind="ExternalInput").ap()
    wq = nc.dram_tensor("wq", [HPC, 128, KC, HD], MM, kind="ExternalInput").ap()
    wk = nc.dram_tensor("wk", [128, KC, HD], MM, kind="ExternalInput").ap()
    wv = nc.dram_tensor("wv", [128, KC, HD], MM, kind="ExternalInput").ap()
    wo = nc.dram_tensor("wo", [NO, 128, H, 512], MM, kind="ExternalInput").ap()
    CS = BF16 if MM == BF16 else F32
    cosT = nc.dram_tensor("cosT", [HD // 2, S], CS, kind="ExternalInput").ap()
    sinT = nc.dram_tensor("sinT", [HD // 2, S], CS, kind="ExternalInput").ap()
    out = nc.dram_tensor("out", [R, D], F32, kind="ExternalOutput").ap()

    with tile.TileContext(nc) as tc, ExitStack() as ctx:
        dram = ctx.enter_context(tc.tile_pool(name="dram", bufs=1, space="DRAM"))
        a2a_in = [dram.tile([NC, 128, R], MM, tag=f"ain{h}", name=f"ain{h}")
                  for h in range(HPC)]
        a2a_out = [dram.tile([NC, 128, R], MM, tag=f"aout{h}", name=f"aout{h}")
                   for h in range(HPC)]
        # tiny warmup collective: absorbs the ~50us first-collective firmware
        # cost while the PE streams QKV, so the real a2a chain runs at ~13us
        warm_i = dram.tile([NC, 64], MM, tag="warm_i", name="warm_i")
        warm_o = dram.tile([NC, 64], MM, tag="warm_o", name="warm_o")

        qkvp = ctx.enter_context(tc.tile_pool(name="qkv", bufs=1))
        QT = [qkvp.tile([128, S], MM, tag=f"qt{h}", name=f"QT{h}")
              for h in range(HPC)]
        KT = qkvp.tile([128, S], MM, tag="kt", name="KT")
        Vn = qkvp.tile([128, S], MM, tag="vn", name="Vn")
        ones = qkvp.tile([128, 128], MM, tag="ones", name="ones")
        masks = [qkvp.tile([128, NB], MM, tag=f"mask{dd}", name=f"mask{dd}")
                 for dd in range(NKJ)]
        with tc.tile_pool(name="init", bufs=1) as initp:
            ones_f = initp.tile([128, 128], F32, tag="ones_f", name="ones_f")
            nc.vector.memset(ones_f, 1.0)
            nc.vector.tensor_copy(ones, ones_f)
            # causal masks for the 4 diagonal offsets (DVE multiplies, bf16)
            onb_f = initp.tile([128, NB], F32, tag="onb_f", name="onb_f")
            onb = initp.tile([128, NB], MM, tag="onb", name="onb")
            nc.vector.memset(onb_f, 1.0)
            nc.vector.tensor_copy(onb, onb_f)
            for dd in range(NKJ):
                nc.gpsimd.affine_select(
                    out=masks[dd], in_=onb, compare_op=mybir.AluOpType.is_ge,
                    fill=0.0, base=-dd * 128, pattern=[[1, NB]],
                    channel_multiplier=-1,
                )
        nc.gpsimd.collective_compute(
            "AllToAll",
            mybir.AluOpType.bypass,
            replica_groups=[list(range(NC))],
            ins=[warm_i.opt()],
            outs=[warm_o.opt()],
        )

        # ============= Phase 1: QKV projection + RoPE =============
        with (
            tc.tile_pool(name="wgt", bufs=1) as wgt,
            tc.tile_pool(name="trig", bufs=1) as trig,
            tc.tile_pool(name="xtp", bufs=8) as xtp,
            tc.tile_pool(name="vt_sb", bufs=1) as vtp,
            tc.tile_pool(name="pj_ps", bufs=2, space="PSUM") as pjps,
            tc.tile_pool(name="tps", bufs=2, space="PSUM") as tps,
            tc.tile_pool(name="rope_t", bufs=4) as ropep,
        ):
            hw_ = HD // 2
            cos_sb = trig.tile([hw_, S], CS, tag="cos", name="cos_sb")
            sin_sb = trig.tile([hw_, S], CS, tag="sin", name="sin_sb")
            ident = trig.tile([128, 128], MM, tag="ident", name="ident")
            make_identity(nc, ident)

            wq_sb = wgt.tile([128, HPC, KC, HD], MM, tag="wq", name="wq_sb")
            wk_sb = wgt.tile([128, KC, HD], MM, tag="wk", name="wk_sb")
            wv_sb = wgt.tile([128, KC, HD], MM, tag="wv", name="wv_sb")
            # first weights split across two cold DMA queues so the first
            # projection matmuls start as early as possible
            nc.scalar.dma_start(out=wk_sb[:, :KC // 2], in_=wk[:, :KC // 2])
            nc.scalar.dma_start(out=wk_sb[:, KC // 2:], in_=wk[:, KC // 2:])
            nc.scalar.dma_start(out=wv_sb, in_=wv)
            nc.scalar.dma_start(out=wq_sb[:, 0], in_=wq[0])
            nc.scalar.dma_start(out=cos_sb, in_=cosT)
            nc.scalar.dma_start(out=sin_sb, in_=sinT)
            for h in range(1, HPC):
                nc.scalar.dma_start(out=wq_sb[:, h], in_=wq[h])

            VT_sb = vtp.tile([128, S], MM, tag="vt", name="VT_sb")

            outs = [(lambda k: wk_sb[:, k, :], "k", 0),
                    (lambda k: wv_sb[:, k, :], "v", 0)]
            for h in range(HPC):
                outs.append((lambda k, h=h: wq_sb[:, h, k, :], "q", h))

            KH = KC // 4
            for n in range(NSB):
                nsl = slice(n * NB, (n + 1) * NB)
                xh = []
                for half in range(4):
                    xb = xtp.tile([128, KH, NB], MM, tag="x", name="xb")
                    if n == 0 and half == 0:
                        # split the very first x chunk so the first matmul
                        # (which only needs k-chunk 0) starts ~4us earlier
                        # on the cold DMA queue
                        h2_ = KH // 2
                        nc.sync.dma_start(out=xb[:, :h2_],
                                          in_=xT[n, :, :h2_, :])
                        nc.sync.dma_start(out=xb[:, h2_:],
                                          in_=xT[n, :, h2_:KH, :])
                    else:
                        nc.sync.dma_start(
                            out=xb, in_=xT[n, :, half * KH:(half + 1) * KH, :])
                    xh.append(xb)
                # output-major with a rotating 3-deep PSUM tag: long
                # same-bank runs keep the PE warm while earlier outputs drain
                for wsel, kind, h in outs:
                    ps = pjps.tile([128, NB], F32, tag="pj", name="ps")
                    for k in range(KC):
                        nc.tensor.matmul(
                            ps, lhsT=wsel(k),
                            rhs=xh[k // KH][:, k % KH, :],
                            start=(k == 0), stop=(k == KC - 1),
                        )
                    if kind == "v":
                        nc.scalar.copy(VT_sb[:, nsl], ps)
                        for t in range(n * NB // 128, (n + 1) * NB // 128):
                            tsl = slice(t * 128, (t + 1) * 128)
                            pst = tps.tile([128, 128], MM, tag="tp",
                                           name="pst")
                            nc.tensor.transpose(pst, VT_sb[:, tsl], ident)
                            nc.scalar.copy(Vn[:, tsl], pst)
                    else:
                        dst = KT if kind == "k" else QT[h]
                        E, O = ps[0:hw_, :], ps[hw_:2 * hw_, :]
                        c, s = cos_sb[:, nsl], sin_sb[:, nsl]
                        t1 = ropep.tile([hw_, NB], F32, tag="rt", name="t1")
                        t2 = ropep.tile([hw_, NB], F32, tag="rt", name="t2")
                        nc.vector.tensor_mul(t1, E, c)
                        nc.vector.tensor_mul(t2, O, s)
                        nc.vector.tensor_sub(dst[0:hw_, nsl], t1, t2)
                        t3 = ropep.tile([hw_, NB], F32, tag="rt", name="t3")
                        t4 = ropep.tile([hw_, NB], F32, tag="rt", name="t4")
                        nc.vector.tensor_mul(t3, E, s)
                        nc.vector.tensor_mul(t4, O, c)
                        nc.vector.tensor_add(dst[hw_:, nsl], t3, t4)


        # P3 SBUF pools open before P2 so their DMAs can prefetch during it
        pcp = ctx.enter_context(tc.tile_pool(name="pc_sb", bufs=1))
        wop = ctx.enter_context(tc.tile_pool(name="wo_sb", bufs=4))
        obp = ctx.enter_context(tc.tile_pool(name="ob_sb", bufs=3))
        piece = {}

        wo_pre = {}

        def get_wob(no):
            if no in wo_pre:
                return wo_pre[no]
            wob = wop.tile([128, H, 512], MM, tag="wob", name="wob")
            q2 = H // 2
            for qq in range(0, H, q2):
                nc.scalar.dma_start(out=wob[:, qq:qq + q2],
                                    in_=wo[no, :, qq:qq + q2])
            wo_pre[no] = wob
            return wob

        # prefetch the first 3 wo blocks: the ACT queue reaches these DMAs
        # before the phase-2 exps, so they stream during attention
        for no in range(PRE_N):
            get_wob(no)

        # ============= Phase 2: causal flash attention =============
        with (
            tc.tile_pool(name="sc_ps", bufs=2, space="PSUM") as scps,
            tc.tile_pool(name="o_ps", bufs=2, space="PSUM") as ops_,
            tc.tile_pool(name="rs_ps", bufs=2, space="PSUM") as rsps,
            tc.tile_pool(name="exp_sb", bufs=3) as exps,
            tc.tile_pool(name="att_sb", bufs=4) as atts,
        ):
            # unified cross-block software pipeline: PV pairs trail the score
            # stream by 2 pairs ACROSS block boundaries, and each block's
            # rowsum matmul + normalize + piece stores are emitted once its
            # last PV pops — so the PE never drains at a block boundary
            inflight = []

            def finish_block(b):
                rcp = atts.tile([128, NB], F32, tag="rcp", name="rcp")
                nc.vector.reciprocal_approx_fast(rcp, b["rs_ps"])
                o_sb = atts.tile([128, NB], MM, tag="osb", name="o_sb")
                nc.vector.tensor_mul(o_sb, b["o_ps"], rcp)
                hh, nn = b["h"], b["n"]
                for jj in range(PIECES):
                    piece_idx = nn * PIECES + jj
                    nc.sync.dma_start(
                        out=a2a_in[hh][piece_idx],
                        in_=o_sb[:, jj * R:(jj + 1) * R],
                    )
                if nn == NSB - 1:
                    nc.gpsimd.collective_compute(
                        "AllToAll",
                        mybir.AluOpType.bypass,
                        replica_groups=[list(range(NC))],
                        ins=[a2a_in[hh].opt()],
                        outs=[a2a_out[hh].opt()],
                    )
                    # pull head hh-1's redistributed pieces into SBUF now;
                    # its a2a completed during head hh's attention
                    if hh >= 1:
                        for i in range(NC):
                            t = pcp.tile([128, R], MM, tag=f"pc{hh - 1}_{i}",
                                         name=f"pc{hh - 1}_{i}")
                            nc.sync.dma_start(out=t, in_=a2a_out[hh - 1][i])
                            piece[(hh - 1, i)] = t

            def emit_pv(entry):
                pair, b = entry
                for (j, pex) in pair:
                    psl = slice(j * 128, (j + 1) * 128)
                    d = j - b["n"] * NKJ
                    last = j == b["last_j"]
                    if d > 0:
                        # diagonal tile: query cols < d*128 are fully masked
                        # (pex is 0 there) — skip streaming them
                        qs = d * 128
                        nc.tensor.matmul(b["o_ps"][:, qs:],
                                         lhsT=Vn[:, psl], rhs=pex[:, qs:],
                                         start=False, stop=last)
                        nc.tensor.matmul(b["rs_ps"][:, qs:], lhsT=ones,
                                         rhs=pex[:, qs:],
                                         start=False, stop=last)
                    else:
                        nc.tensor.matmul(b["o_ps"], lhsT=Vn[:, psl], rhs=pex,
                                         start=(j == 0), stop=last)
                        nc.tensor.matmul(b["rs_ps"], lhsT=ones, rhs=pex,
                                         start=(j == 0), stop=last)
                b["npv"] -= 1
                if b["npv"] == 0:
                    finish_block(b)

            def pump(maxlag):
                while len(inflight) > maxlag:
                    emit_pv(inflight.pop(0))

            for h in range(HPC):
                for n in range(NSB):
                    nsl = slice(n * NB, (n + 1) * NB)
                    nkj = (n + 1) * NKJ
                    b = dict(
                        o_ps=ops_.tile([128, NB], F32, tag="o", name="o_ps"),
                        rs_ps=rsps.tile([128, NB], F32, tag="rs",
                                        name="rs_ps"),
                        h=h, n=n, last_j=nkj - 1, npv=nkj // 2,
                    )
                    q_rhs = QT[h][:, nsl]

                    for jp in range(nkj // 2):
                        sc = scps.tile([128, 2, NB], F32, tag="sc",
                                       name="sc")
                        ex = exps.tile([128, 2, NB], MM, tag="ex", name="ex")
                        pair = []
                        for half in range(2):
                            j = 2 * jp + half
                            jsl = slice(j * 128, (j + 1) * 128)
                            d = j - n * NKJ
                            if d > 0:
                                # diagonal tile: query cols < d*128 are
                                # fully masked downstream — skip them in the
                                # score stream too
                                qs = d * 128
                                nc.tensor.matmul(sc[:, half, qs:],
                                                 lhsT=KT[:, jsl],
                                                 rhs=q_rhs[:, qs:],
                                                 start=True, stop=True)
                            else:
                                nc.tensor.matmul(sc[:, half], lhsT=KT[:, jsl],
                                                 rhs=q_rhs, start=True,
                                                 stop=True)
                            pair.append((j, ex[:, half]))
                        pump(2)
                        if 2 * jp + 1 - n * NKJ <= 0:
                            nc.scalar.activation(
                                ex.rearrange("p a b -> p (a b)"),
                                sc.rearrange("p a b -> p (a b)"),
                                mybir.ActivationFunctionType.Exp, scale=scale,
                            )
                        else:
                            # sliced exps so the dead region of sc (stale
                            # PSUM) is never read; the dead region of ex is
                            # never read either (PV/rowsum skip it)
                            for half in range(2):
                                d = 2 * jp + half - n * NKJ
                                qs = d * 128 if d > 0 else 0
                                nc.scalar.activation(
                                    ex[:, half, qs:], sc[:, half, qs:],
                                    mybir.ActivationFunctionType.Exp,
                                    scale=scale,
                                )
                        for half in range(2):
                            j = 2 * jp + half
                            d = j - n * NKJ
                            if d >= 0:  # overlaps the causal diagonal
                                # only the 128-wide strip crossing the
                                # diagonal is partial; it matches masks[0]'s
                                # leading strip for every offset d
                                qs = d * 128
                                nc.vector.tensor_mul(
                                    ex[:, half, qs:qs + 128],
                                    ex[:, half, qs:qs + 128],
                                    masks[0][:, :128])
                        inflight.append((pair, b))
            pump(0)

            for i in range(NC):
                t = pcp.tile([128, R], MM, tag=f"pc{HPC - 1}_{i}",
                             name=f"pc{HPC - 1}_{i}")
                nc.sync.dma_start(out=t, in_=a2a_out[HPC - 1][i])
                piece[(HPC - 1, i)] = t


        # ============= Phase 3: out-projection =============
        with tc.tile_pool(name="op_ps", bufs=2, space="PSUM") as opps:

            def emit_mms(pso, wob, heads, first):
                for m in range(MT):
                    f = first
                    for h in heads:
                        for i in range(NC):
                            g = i * HPC + h
                            last = (h == HPC - 1) and (i == NC - 1)
                            nc.tensor.matmul(
                                pso[m],
                                lhsT=piece[(h, i)][:, m * 128:(m + 1) * 128],
                                rhs=wob[:, g, :], start=f, stop=last,
                            )
                            f = False
                return False

            def drain(pso, no):
                osl = slice(no * 512, (no + 1) * 512)
                for m in range(MT):
                    ob = obp.tile([128, 512], F32, tag="ob", name="ob")
                    nc.vector.tensor_copy(ob, pso[m])
                    nc.sync.dma_start(
                        out=out[m * 128:(m + 1) * 128, osl], in_=ob)

            # wave schedule over the first DEFER no-blocks: heads {0,1}
            # (pair-0 a2a, done mid-attention) first, then h2 and h3 whose
            # pair-1 a2a lands while the h01 wave streams
            DEFER = min(NO, 4) if HPC > 1 else 0
            state = {}
            for no in range(DEFER):
                wob = get_wob(no)
                pso = [opps.tile([128, 512], F32, tag=f"po{m}", name=f"pso{m}",
                                 bufs=max(2, DEFER)) for m in range(MT)]
                state[no] = (pso, wob)
                emit_mms(pso, wob, [0, 1], True)
            for no in range(DEFER):
                pso, wob = state[no]
                emit_mms(pso, wob, [2], False)
            for no in range(DEFER):
                pso, wob = state[no]
                emit_mms(pso, wob, [3], False)
                drain(pso, no)
            for no in range(DEFER, NO):
                wob = get_wob(no)
                pso = [opps.tile([128, 512], F32, tag=f"po{m}", name=f"pso{m}",
                                 bufs=max(2, DEFER)) for m in range(MT)]
                emit_mms(pso, wob, range(HPC), True)
                drain(pso, no)

    nc.compile()
    return nc


def prep_inputs(cfg, x, wq, wk, wv, wo, freqs_cos, freqs_sin):
    """Host-side sharding/layout prep. Returns list of per-core input dicts."""
    S, D, H, KVH, HD, NC = (cfg["S"], cfg["D"], cfg["H"], cfg["KVH"], cfg["HD"],
                            cfg["n_cores"])
    HPC = H // NC
    if cfg.get("dt", "bf16") == "bf16":
        import ml_dtypes
        mdt = ml_dtypes.bfloat16
    else:
        mdt = np.float32
    x = np.asarray(x, np.float32).reshape(S, D)
    wq = np.asarray(wq, np.float32)
    wk = np.asarray(wk, np.float32)
    wv = np.asarray(wv, np.float32)
    wo = np.asarray(wo, np.float32)
    cos = np.asarray(freqs_cos, np.float32)
    sin = np.asarray(freqs_sin, np.float32)

    NSB, NB = S // cfg["NB"], cfg["NB"]
    KC = D // 128
    NO = D // 512
    # x blocked: xp[n, p, k, s] = x[n*NB+s, k*128+p]
    xp = np.ascontiguousarray(
        x.reshape(NSB, NB, KC, 128).transpose(0, 3, 2, 1)).astype(mdt)
    cosT = np.ascontiguousarray(cos.T).astype(mdt)      # [HD/2, S]
    sinT = np.ascontiguousarray(sin.T).astype(mdt)
    # wo blocked: wop_[no, p, g, m] = wo[g*128+p, no*512+m]
    wo_c = np.ascontiguousarray(
        wo.reshape(H, 128, NO, 512).transpose(2, 1, 0, 3)).astype(mdt)

    # de-interleave rope pairs: new col i <- 2i, new col i+HD/2 <- 2i+1
    idx = np.concatenate([np.arange(0, HD, 2), np.arange(1, HD, 2)])
    wq_p = wq.reshape(D, H, HD)[:, :, idx]
    wk_p = wk.reshape(D, KVH, HD)[:, :, idx]
    wv_r = wv.reshape(D, KVH, HD)

    def wblock(w):
        # [D, M] -> [128, KC, M] with wb[p, k, m] = w[k*128+p, m]
        M = w.shape[1]
        return np.ascontiguousarray(
            w.reshape(KC, 128, M).transpose(1, 0, 2)).astype(mdt)

    in_maps = []
    for c in range(NC):
        kv = c * KVH // NC
        in_maps.append(dict(
            xT=xp,
            wq=np.stack([wblock(np.ascontiguousarray(wq_p[:, c * HPC + h]))
                         for h in range(HPC)]),
            wk=wblock(np.ascontiguousarray(wk_p[:, kv])),
            wv=wblock(np.ascontiguousarray(wv_r[:, kv])),
            wo=wo_c,
            cosT=cosT,
            sinT=sinT,
        ))
    return in_maps


_CACHED = {}


def _get_nc(cfg_key=None):
    if "nc" not in _CACHED:
        _CACHED["nc"] = build_bass(FULL_CFG)
    return _CACHED["nc"]


def run_spmd(x, wq, wk, wv, wo, freqs_cos, freqs_sin, **spmd_kwargs):
    """Build (cached), run on 8 cores, return (full_output, BassKernelResults)."""
    from concourse.bass_utils import run_bass_kernel_spmd

    cfg = FULL_CFG
    NC = cfg["n_cores"]
    in_maps = prep_inputs(cfg, x, wq, wk, wv, wo, freqs_cos, freqs_sin)
    nc = _get_nc()
    res = run_bass_kernel_spmd(nc, in_maps, list(range(NC)), **spmd_kwargs)
    parts = [res.results[c]["out"] for c in range(NC)]
    full = np.concatenate(parts, axis=0)
    return full.reshape(1, cfg["S"], cfg["D"]).astype(np.float32), res


def kernel(x, wq, wk, wv, wo, freqs_cos, freqs_sin):
    out, _ = run_spmd(x, wq, wk, wv, wo, freqs_cos, freqs_sin)
    return out

